# revision 51
# baseline (speedup 1.0000x reference)
"""Trainium2 Bass kernel for nn_Attention_1898375545286 (sparse/triangle attention).

Per pair-row n (256 of them, 32 per core x 8 cores):
  q = (q_x[n] @ Wq)/sqrt(32), k = kv_x[n] @ Wk, v = kv_x[n] @ Wv  (heads of 32)
  a = softmax_k(q.k + mask_bias[n,k] + tri_bias[h,q,k])
  out[n] = ((a @ v) * sigmoid(q_x[n] @ Wg)) @ Wo

Device dataflow (everything "transposed": hc/c on partitions, q on free):
  - host pre-transposes q_x/kv_x to [n, c, q] bf16; all matmuls bf16
  - gate phase 0: gT projections batched 4 rows/psum-tile, exp(-g) in 8 big
    ACT calls into a resident ag_all bf16 buffer
  - attention computed as a^T [k, q]: QK via row-tiled (K=32) packed matmuls
    accumulating onto tri_bias pre-written to PSUM via identity matmuls
  - softmax denominator via column-tiled ones-matmul broadcasting the per-head
    sum across the head's 32 partitions; normalization folded into the gate:
    o * sigmoid(g) / s == o / (s * (1 + exp(-g)))
  - 3-stage software pipeline: step i emits OUT(i-2) | ATTN(i-1) | PROJ(i) so
    each engine always has independent work from a neighboring row
  - PSUM (8 banks): pp [128,1024] bufs=1 (qT|kT|v), at [128,1024] bufs=2
    (a^T head-halves), oo [128,1024] bufs=1 (per-pair so2|oT2, then op2
    overlays dead so2; also hosts the phase-0 gate projection tiles)
"""
import sys

sys.path.insert(0, "/opt/trn_rl_repo")

import math

import numpy as np
import ml_dtypes

N_CORES = 8
B, N, Q, C = 1, 256, 256, 128
H, C_HID = 4, 32
ROWS = N // N_CORES  # rows per core

_cache = {}

TRACE = False
LAST_RESULTS = None


def _build(mask_zero=True):
    import concourse.bass as bass
    import concourse.tile as tile
    from concourse import mybir, bacc

    f32 = mybir.dt.float32
    bf16 = mybir.dt.bfloat16
    Exp = mybir.ActivationFunctionType.Exp

    nc = bacc.Bacc("TRN2", target_bir_lowering=False, debug=False,
                   num_devices=N_CORES)

    G = 8  # rows per DMA batch
    NB = ROWS // G
    qxT = nc.dram_tensor("qxT", [NB, C, G * Q], bf16, kind="ExternalInput").ap()
    kxT = nc.dram_tensor("kxT", [NB, C, G * Q], bf16, kind="ExternalInput").ap()
    tri = nc.dram_tensor("tri", [128, 2 * H * Q], bf16, kind="ExternalInput").ap()


    wq = nc.dram_tensor("wq", [C, C], bf16, kind="ExternalInput").ap()
    wk = nc.dram_tensor("wk", [C, C], bf16, kind="ExternalInput").ap()
    wv = nc.dram_tensor("wv", [C, C], bf16, kind="ExternalInput").ap()
    wg = nc.dram_tensor("wg", [C, C], bf16, kind="ExternalInput").ap()
    wo = nc.dram_tensor("wo", [C, C], bf16, kind="ExternalInput").ap()
    eye = nc.dram_tensor("eye", [C, C], bf16, kind="ExternalInput").ap()
    ones32 = nc.dram_tensor("ones32", [128, 32], bf16, kind="ExternalInput").ap()
    if not mask_zero:
        maskd = nc.dram_tensor("maskd", [128, ROWS, 2], f32,
                               kind="ExternalInput").ap()
    out_d = nc.dram_tensor("out", [NB, 128, G * 256], f32,
                           kind="ExternalOutput").ap()

    with tile.TileContext(nc) as tc:
        with tc.tile_pool(name="const", bufs=1) as cpool, \
             tc.tile_pool(name="projsb", bufs=4) as ppool, \
             tc.tile_pool(name="aexp", bufs=4) as epool, \
             tc.tile_pool(name="gate", bufs=2) as gpool, \
             tc.tile_pool(name="ostp", bufs=2) as opool, \
             tc.tile_pool(name="pp_ps", bufs=1, space="PSUM") as pp_pool, \
             tc.tile_pool(name="oo_ps", bufs=1, space="PSUM") as oo_pool, \
             tc.tile_pool(name="at_ps", bufs=2, space="PSUM") as at_pool:

            tri_sb = cpool.tile([128, 2 * H * Q], bf16)


            wq_sb = cpool.tile([C, C], bf16, tag="wq")
            wk_sb = cpool.tile([C, C], bf16, tag="wk")
            wv_sb = cpool.tile([C, C], bf16, tag="wv")
            wg_sb = cpool.tile([C, C], bf16, tag="wg")
            wo_sb = cpool.tile([C, C], bf16, tag="wo")
            eye_sb = cpool.tile([C, C], bf16, tag="eye")
            ones_sb = cpool.tile([128, 32], bf16, tag="ones")
            nc.sync.dma_start(out=tri_sb[:], in_=tri[:])
            nc.sync.dma_start(out=wq_sb[:], in_=wq[:])
            nc.sync.dma_start(out=wk_sb[:], in_=wk[:])
            nc.sync.dma_start(out=wv_sb[:], in_=wv[:])
            nc.sync.dma_start(out=wg_sb[:], in_=wg[:])
            nc.sync.dma_start(out=wo_sb[:], in_=wo[:])
            nc.sync.dma_start(out=eye_sb[:], in_=eye[:])
            nc.sync.dma_start(out=ones_sb[:], in_=ones32[:])
            if not mask_zero:
                mask_sb = cpool.tile([128, ROWS, 2], f32, tag="mask")
                nc.sync.dma_start(out=mask_sb[:], in_=maskd[:])

            # resident inputs + gate buffer, split per-batch so dependency
            # tracking stays fine-grained (readers only wait their batch)
            # resident inputs + gate buffer, split per-batch so dependency
            # tracking stays fine-grained (readers only wait their batch)
            qx_b, kx_b, ag_b = [], [], []
            for b in range(NB):
                qx_b.append(cpool.tile([C, G * Q], bf16, name=f"qxb{b}"))
                kx_b.append(cpool.tile([C, G * Q], bf16, name=f"kxb{b}"))
                nc.sync.dma_start(out=qx_b[b][:], in_=qxT[b])
                nc.sync.dma_start(out=kx_b[b][:], in_=kxT[b])
            for g in range(ROWS // 4):
                ag_b.append(cpool.tile([C, 4 * Q], bf16, name=f"agb{g}"))

            def qx_row(i):
                return qx_b[i // G][:, (i % G) * Q:(i % G + 1) * Q]

            def kx_row(i):
                return kx_b[i // G][:, (i % G) * Q:(i % G + 1) * Q]

            # ---- gate: gT proj 4 rows/psum tile, big exp calls. Runs on the
            # oo pool (idle during the pipeline ramp), interleaved with the
            # first main-loop steps.
            def emit_gate(gb):
                gps = oo_pool.tile([128, 1024], f32, tag="oo",
                                   name=f"gps{gb}")
                b, off = divmod(gb * 1024, G * Q)
                for half in range(2):
                    nc.tensor.matmul(
                        gps[:, half * 512:(half + 1) * 512],
                        lhsT=wg_sb[:],
                        rhs=qx_b[b][:, off + half * 512:off + (half + 1) * 512],
                        start=True, stop=(half == 1),
                        skip_group_check=True)
                nc.scalar.activation(ag_b[gb][:], gps[:], Exp, scale=-1.0)

            # ---- 3-stage pipelined main loop (out-stage batched per row PAIR)
            qkv_sbs = {}     # i -> qkv sbuf tile
            aexps = {}       # (pair, half) -> aexp2 sbuf tile [r2, hh, kc, q]
            ost_t = {"t": None}

            def emit_proj(i):
                qx_sb = qx_row(i)
                kx_sb = kx_row(i)
                pp = pp_pool.tile([128, 1024], f32, tag="pp")
                nc.tensor.matmul(pp[:, 0:256], lhsT=wq_sb[:], rhs=qx_sb,
                                 start=True, stop=False, skip_group_check=True)
                nc.tensor.matmul(pp[:, 256:512], lhsT=wk_sb[:], rhs=kx_sb,
                                 start=False, stop=False, skip_group_check=True)
                for kc in range(2):
                    # kc==0 is the first write to pp's second bank
                    nc.tensor.matmul(pp[:, 512 + kc * 128:512 + (kc + 1) * 128],
                                     lhsT=kx_sb[:, kc * 128:(kc + 1) * 128],
                                     rhs=wv_sb[:], start=(kc == 0),
                                     stop=(kc == 1), skip_group_check=True)
                qkv_sb = ppool.tile([C, 768], bf16, tag="qkv")
                nc.vector.tensor_copy(out=qkv_sb[:], in_=pp[:, 0:768])
                qkv_sbs[i] = qkv_sb

            def emit_attn(i):
                qkv_sb = qkv_sbs[i]
                qT_sb = qkv_sb[:, 0:256]
                kT_sb = qkv_sb[:, 256:512]
                for half in range(2):
                    at = at_pool.tile([128, 1024], f32, tag="at")
                    for hh in range(2):
                        h = half * 2 + hh
                        nc.tensor.matmul(at[:, hh * 512:hh * 512 + 512],
                                         lhsT=eye_sb[:],
                                         rhs=tri_sb[:, h * 512:(h + 1) * 512],
                                         start=True, stop=False,
                                         skip_group_check=True)
                    for kc in range(2):
                        for hh in range(2):
                            h = half * 2 + hh
                            s = hh * 512 + kc * 256
                            nc.tensor.matmul(
                                at[:, s:s + Q],
                                lhsT=kT_sb[32 * h:32 * (h + 1),
                                           kc * 128:(kc + 1) * 128],
                                rhs=qT_sb[32 * h:32 * (h + 1), :],
                                start=False, stop=(kc == 1),
                                tile_position=(32 * h, 0),
                                skip_group_check=True)
                    j, r2 = divmod(i, 2)
                    if r2 == 0:
                        aexps[(j, half)] = epool.tile([128, 2048], bf16,
                                                      tag="aexp",
                                                      name=f"aexp{j}_{half}")
                    dst = aexps[(j, half)][:, r2 * 1024:(r2 + 1) * 1024]
                    if mask_zero:
                        nc.scalar.activation(dst, at[:], Exp)
                    else:
                        av = dst.rearrange("p (hh k q) -> p hh k q",
                                           hh=2, k=2)
                        iv = at[:].rearrange("p (hh k q) -> p hh k q",
                                             hh=2, k=2)
                        for kc in range(2):
                            nc.scalar.activation(av[:, :, kc, :],
                                                 iv[:, :, kc, :],
                                                 Exp, bias=mask_sb[:, i, kc])

            def emit_out_pair(j):
                # rows i0 = 2j, i0+1; so2/oT2/op2 laid out [r2, q] (512 wide)
                i0 = 2 * j
                oo = oo_pool.tile([128, 1024], f32, tag="oo")
                so2 = oo[:, 0:512]
                oT2 = oo[:, 512:1024]
                op2 = oo[:, 0:512]  # overlays so2 after it is consumed
                for half in range(2):
                    aexp2 = aexps.pop((j, half))
                    a4 = aexp2[:].rearrange("p (r hh kc q) -> p hh kc r q",
                                            r=2, hh=2, kc=2)
                    for kc in range(2):
                        for hh in range(2):
                            h = half * 2 + hh
                            nc.tensor.matmul(so2[32 * h:32 * (h + 1), :],
                                             lhsT=ones_sb[:],
                                             rhs=a4[:, hh, kc, :, :],
                                             start=(kc == 0), stop=(kc == 1),
                                             tile_position=(0, 32 * h),
                                             skip_group_check=True)
                    for r2 in range(2):
                        v_sb = qkv_sbs[i0 + r2][:, 512:768]
                        for kc in range(2):
                            for hh in range(2):
                                h = half * 2 + hh
                                s = r2 * 1024 + hh * 512 + kc * 256
                                nc.tensor.matmul(
                                    oT2[32 * h:32 * (h + 1),
                                        r2 * 256:(r2 + 1) * 256],
                                    lhsT=v_sb[:, kc * 128 + 32 * h:
                                              kc * 128 + 32 * (h + 1)],
                                    rhs=aexp2[:, s:s + Q],
                                    start=(kc == 0), stop=(kc == 1),
                                    tile_position=(0, 32 * h),
                                    skip_group_check=True)
                qkv_sbs.pop(i0)
                qkv_sbs.pop(i0 + 1)
                u2 = gpool.tile([C, 512], f32, tag="u2")
                ge = gpool.tile([C, 512], f32, tag="ge")
                of_sb = gpool.tile([C, 512], bf16, tag="of")
                ag2 = ag_b[j // 2][:, (j % 2) * 512:(j % 2 + 1) * 512]
                nc.vector.scalar_tensor_tensor(
                    out=u2[:], in0=ag2, scalar=1.0,
                    in1=so2, op0=mybir.AluOpType.add, op1=mybir.AluOpType.mult)
                nc.vector.reciprocal_approx_fast(out=ge[:], in_=u2[:])
                nc.vector.tensor_tensor(out=of_sb[:], in0=oT2, in1=ge[:],
                                        op=mybir.AluOpType.mult)
                for r2 in range(2):
                    for qc in range(2):
                        s = r2 * 256 + qc * 128
                        nc.tensor.matmul(op2[:, s:s + 128],
                                         lhsT=of_sb[:, s:s + 128],
                                         rhs=wo_sb[:],
                                         start=(r2 == 0 and qc == 0),
                                         stop=(r2 == 1 and qc == 1),
                                         skip_group_check=True)
                b, jj = divmod(j, G // 2)
                if jj == 0:
                    ost_t["t"] = opool.tile([128, G * 256], f32, tag="ost",
                                            name=f"ost{b}")
                nc.vector.tensor_copy(
                    out=ost_t["t"][:, jj * 512:(jj + 1) * 512], in_=op2)
                # fire the out DMA per pair so output drains incrementally
                # and the kernel tail only waits on the last 256KB
                nc.sync.dma_start(out=out_d[b, :, jj * 512:(jj + 1) * 512],
                                  in_=ost_t["t"][:, jj * 512:(jj + 1) * 512])

            for step in range(ROWS + 3):
                if step % 2 == 0 and step // 2 < ROWS // 4:
                    emit_gate(step // 2)
                if step >= 3 and step % 2 == 1:
                    emit_out_pair((step - 3) // 2)
                if 0 <= step - 1 < ROWS:
                    emit_attn(step - 1)
                if step < ROWS:
                    emit_proj(step)
    nc.compile()
    return nc


def _host_prep(inputs):
    q_x = np.ascontiguousarray(inputs["q_x"], np.float32)[0]    # [N, Q, C]
    kv_x = np.ascontiguousarray(inputs["kv_x"], np.float32)[0]
    tri_b = np.asarray(inputs["tri_bias"], np.float32)[0, 0]    # [H, Q, K]
    mask_b = np.asarray(inputs["mask_bias"], np.float32)[0, :, 0, 0, :]  # [N, K]
    Wq = np.asarray(inputs["Wq"], np.float32) / math.sqrt(C_HID)
    Wk = np.asarray(inputs["Wk"], np.float32)
    Wv = np.asarray(inputs["Wv"], np.float32)
    Wg = np.asarray(inputs["Wg"], np.float32)
    Wo = np.asarray(inputs["Wo"], np.float32)

    # batched layout: [N/8, C, 8*Q]; arr[b, c, r*Q+q] = x[8b+r, q, c]
    def batch_T(x):
        return np.ascontiguousarray(
            x.reshape(N // 8, 8, Q, C).transpose(0, 3, 1, 2)
            .reshape(N // 8, C, 8 * Q).astype(ml_dtypes.bfloat16))
    qxT = batch_T(q_x)
    kxT = batch_T(kv_x)

    # tri layout: [128, (h, kc, q)]; tri[p, (h*2+kc)*Q + q] = tri_b[h, q, kc*128+p]
    tri_dev = np.empty((128, 2 * H * Q), np.float32)
    for h in range(H):
        for kc in range(2):
            s = (h * 2 + kc) * Q
            tri_dev[:, s:s + Q] = tri_b[h, :, kc * 128:(kc + 1) * 128].T

    bf = ml_dtypes.bfloat16
    shared = {
        "tri": tri_dev.astype(bf),
        "wq": Wq.astype(bf), "wk": Wk.astype(bf), "wv": Wv.astype(bf),
        "wg": Wg.astype(bf), "wo": Wo.astype(bf),
        "eye": np.eye(C, dtype=bf),
        "ones32": np.ones((128, 32), bf),
    }
    nb = ROWS // 8
    in_maps = []
    for c in range(N_CORES):
        b0 = c * nb
        in_maps.append({
            "qxT": np.ascontiguousarray(qxT[b0:b0 + nb]),
            "kxT": np.ascontiguousarray(kxT[b0:b0 + nb]),
            **shared,
        })
    return in_maps, mask_b


def kernel(**inputs):
    global LAST_RESULTS
    from concourse import bass_utils

    in_maps, mask_b = _host_prep(inputs)
    mask_zero = bool(np.all(mask_b == 0.0))
    if not mask_zero:
        # mask layout [128, rows, kc]: mask[p, n, kc] = mask_b[row, kc*128+p]
        for c in range(N_CORES):
            r0 = c * ROWS
            md = np.empty((128, ROWS, 2), np.float32)
            for kc in range(2):
                md[:, :, kc] = mask_b[r0:r0 + ROWS, kc * 128:(kc + 1) * 128].T
            in_maps[c]["maskd"] = md
    key = ("nc", mask_zero)
    if key not in _cache:
        _cache[key] = _build(mask_zero)
    nc = _cache[key]
    res = bass_utils.run_bass_kernel_spmd(nc, in_maps, list(range(N_CORES)),
                                          trace=TRACE)
    if TRACE:
        LAST_RESULTS = res
    # device layout [NB, 128(qp), 8(r), 2(qc), 128(c)] -> [n, q, c]
    out = np.concatenate([res.results[c]["out"] for c in range(N_CORES)], axis=0)
    out = out.reshape(N // 8, 128, 8, 2, 128).transpose(0, 2, 3, 1, 4)
    return np.ascontiguousarray(out.reshape(B, N, Q, C))
